# revision 1
# baseline (speedup 1.0000x reference)
"""Trainium2 Bass kernel for nn_Long_LSTM_Top (2-window masked LSTM + sum-pool + FC).

Strategy (hardcoded for B=256, T=300, C=128, H=256, CLS=60, windows at p=0 and
p=145, each 154 long, over the lag-1 difference sequence d[p] = x[p+1]-x[p]):

- Data-parallel over batch across 8 cores (32 batch rows/core).
- Per core, both windows' LSTM chains run fused: every tensor keeps the
  feature dim on partitions and (window, row) = 64 columns in the free dim,
  so the recurrence needs no transposes and each weight tile is loaded once
  per step for both windows.
- Scan step w (0..298): psum[128, 8, 64] accumulates, per gate-chunk j,
  xproj = W_ihT.T @ dmask[w]  (start=True)  then += W_hhT.T @ h  (k=0,1).
  Gate order in psum blocks: [g,g,i,i,f,f,o,o] so tanh(g) starts earliest.
- d is pre-masked per window (zeros outside the window) so all 299 steps are
  uniform; window-1's chain computes exact zeros until its window opens.
- Matmul operands fp16 (1 cycle/row on PE; fp32 would be 4), all elementwise
  state math fp32. Final FC in fp32.
"""

import numpy as np

import concourse.bass as bass
import concourse.mybir as mybir
from concourse import bacc
from concourse.tile import TileContext
from concourse.masks import make_identity

F32 = mybir.dt.float32
F16 = mybir.dt.float16

B, T, C, H, CLS = 256, 300, 128, 256, 60
START, STRIDE, WIN = 1, 145, 154
NUM_WIN = 2
L = T - START  # 299
NCORES = 8
BC = B // NCORES  # 32 rows per core
NSTEP = L  # 299 wall steps

# psum block j holds gate chunk CHUNK_ORDER[j] (PyTorch gate order i,f,g,o in
# chunks of 128: i=0,1 f=2,3 g=4,5 o=6,7). Blocks ordered [g,g,i,i,f,f,o,o].
CHUNK_ORDER = [4, 5, 0, 1, 2, 3, 6, 7]


def build(bias_zero: bool = True, nstep: int = NSTEP):
    """Build the per-core Bass module. Returns nc."""
    nc = bacc.Bacc("TRN2", target_bir_lowering=False, debug=False)

    x_d = nc.declare_dram_parameter("x", [BC * T, C], F32, isOutput=False)
    wih_d = nc.declare_dram_parameter("w_ih", [4 * H, C], F32, isOutput=False)
    whh_d = nc.declare_dram_parameter("w_hh", [4 * H, H], F32, isOutput=False)
    wfc_d = nc.declare_dram_parameter("w_fc", [CLS, NUM_WIN * H], F32, isOutput=False)
    bias_d = nc.declare_dram_parameter("bias", [4 * H], F32, isOutput=False)
    out_d = nc.declare_dram_parameter("out", [CLS, BC], F32, isOutput=True)

    with TileContext(nc) as tc:
        with (
            tc.tile_pool(name="persist", bufs=1) as persist,
            tc.tile_pool(name="prep", bufs=3) as prep,
            tc.tile_pool(name="prep_ps", bufs=2, space="PSUM") as prep_ps,
            tc.tile_pool(name="scan_ps", bufs=4, space="PSUM") as scan_ps,
            tc.tile_pool(name="fc_ps", bufs=1, space="PSUM") as fc_ps,
            tc.tile_pool(name="state_h", bufs=3) as state_h,
            tc.tile_pool(name="state_c", bufs=3) as state_c,
            tc.tile_pool(name="acts", bufs=3) as acts,
        ):
            ident = persist.tile([128, 128], F32)
            make_identity(nc, ident)

            # ---- load x and transpose to xT[c, (r t)] --------------------
            xT = persist.tile([128, BC * T], F32)  # col = r*300 + t
            for j in range(75):
                xn = prep.tile([128, 128], F32, tag="xn")
                nc.sync.dma_start(out=xn, in_=x_d[j * 128:(j + 1) * 128, :])
                pt = prep_ps.tile([128, 128], F32)
                nc.tensor.transpose(pt, xn, ident)
                nc.scalar.copy(out=xT[:, j * 128:(j + 1) * 128], in_=pt)

            # ---- masked lag-difference, fp16, layout [c, (w win r)] ------
            dm = persist.tile([128, NSTEP, NUM_WIN, BC], F16)
            nc.vector.memset(dm, 0.0)
            xT3 = xT[:].rearrange("p (r t) -> p r t", r=BC)
            for r in range(BC):
                # window 0 active at p in [0, 154)
                nc.vector.tensor_sub(
                    dm[:, 0:WIN, 0, r],
                    xT3[:, r, 1:WIN + 1],
                    xT3[:, r, 0:WIN],
                )
                # window 1 active at p in [145, 299)
                nc.vector.tensor_sub(
                    dm[:, STRIDE:L, 1, r],
                    xT3[:, r, STRIDE + 1:L + 1],
                    xT3[:, r, STRIDE:L],
                )

            # ---- weights: transpose to [in_dim, gate] fp16 ---------------
            wihT = persist.tile([128, 8 * 128], F16)  # col block = gate chunk
            for g in range(8):
                wn = prep.tile([128, C], F32, tag="wn")
                nc.sync.dma_start(
                    out=wn, in_=wih_d[g * 128:(g + 1) * 128, :]
                )
                pt = prep_ps.tile([128, 128], F32)
                nc.tensor.transpose(pt, wn, ident)
                nc.scalar.copy(out=wihT[:, g * 128:(g + 1) * 128], in_=pt)

            whhT = persist.tile([128, 16 * 128], F16)  # col block = g*2+k
            for g in range(8):
                wn = prep.tile([128, H], F32, tag="wn2")
                nc.sync.dma_start(
                    out=wn, in_=whh_d[g * 128:(g + 1) * 128, :]
                )
                for k in range(2):
                    pt = prep_ps.tile([128, 128], F32)
                    nc.tensor.transpose(pt, wn[:, k * 128:(k + 1) * 128], ident)
                    nc.scalar.copy(
                        out=whhT[:, (g * 2 + k) * 128:(g * 2 + k + 1) * 128], in_=pt
                    )

            wfcT = persist.tile([128, 4 * CLS], F32)  # col block = feat chunk
            wfcn = persist.tile([CLS, NUM_WIN * H], F32)
            nc.sync.dma_start(out=wfcn, in_=wfc_d[:])
            for k in range(4):
                pt = prep_ps.tile([128, 128], F32)
                nc.tensor.transpose(
                    pt[:, :CLS], wfcn[:, k * 128:(k + 1) * 128], ident[:CLS, :CLS]
                )
                nc.scalar.copy(out=wfcT[:, k * CLS:(k + 1) * CLS], in_=pt[:, :CLS])

            bias_sb = None
            if not bias_zero:
                bias_sb = persist.tile([128, 8], F32)
                nc.sync.dma_start(
                    out=bias_sb, in_=bias_d[:].rearrange("(g p) -> p g", p=128)
                )

            # All prep (DMAs on many queues, transposes, masked-d subs) ends
            # here; without this barrier the first scan matmuls accumulate
            # more sync waits than the LDW ISA slot allows.
            tc.strict_bb_all_engine_barrier()

            # ---- scan ----------------------------------------------------
            pooled = persist.tile([128, 2, NUM_WIN * BC], F32)
            nc.vector.memset(pooled, 0.0)
            h_prev = state_h.tile([128, 2, NUM_WIN * BC], F16, tag="h")
            nc.vector.memset(h_prev, 0.0)
            c_prev = state_c.tile([128, 2, NUM_WIN * BC], F32, tag="c")
            nc.vector.memset(c_prev, 0.0)

            sig = mybir.ActivationFunctionType.Sigmoid
            tnh = mybir.ActivationFunctionType.Tanh

            for w in range(nstep):
                ps = scan_ps.tile([128, 8, NUM_WIN * BC], F32, tag="ps")
                rhs_d = dm[:, w, :, :]
                for j in range(8):
                    gc = CHUNK_ORDER[j]
                    nc.tensor.matmul(
                        out=ps[:, j, :],
                        lhsT=wihT[:, gc * 128:(gc + 1) * 128],
                        rhs=rhs_d,
                        start=True,
                        stop=False,
                    )
                    for k in range(2):
                        nc.tensor.matmul(
                            out=ps[:, j, :],
                            lhsT=whhT[:, (gc * 2 + k) * 128:(gc * 2 + k + 1) * 128],
                            rhs=h_prev[:, k, :],
                            start=False,
                            stop=(k == 1),
                        )

                tg = acts.tile([128, 2, NUM_WIN * BC], F32, tag="tg")
                sifo = acts.tile([128, 6, NUM_WIN * BC], F32, tag="sifo")
                if bias_zero:
                    nc.scalar.activation(tg, ps[:, 0:2, :], tnh)
                    nc.scalar.activation(sifo[:, 0:4, :], ps[:, 2:6, :], sig)
                    nc.scalar.activation(sifo[:, 4:6, :], ps[:, 6:8, :], sig)
                else:
                    for j in range(8):
                        dst = tg[:, j, :] if j < 2 else sifo[:, j - 2, :]
                        nc.scalar.activation(
                            dst,
                            ps[:, j, :],
                            tnh if j < 2 else sig,
                            bias=bias_sb[:, CHUNK_ORDER[j]:CHUNK_ORDER[j] + 1],
                        )

                tmp = acts.tile([128, 2, NUM_WIN * BC], F32, tag="tmp")
                nc.vector.tensor_mul(tmp, sifo[:, 0:2, :], tg)  # i*g
                cn = state_c.tile([128, 2, NUM_WIN * BC], F32, tag="c")
                nc.vector.tensor_mul(cn, sifo[:, 2:4, :], c_prev)  # f*c
                nc.vector.tensor_add(cn, cn, tmp)
                tcn = acts.tile([128, 2, NUM_WIN * BC], F32, tag="tc")
                nc.scalar.activation(tcn, cn, tnh)
                hn = state_h.tile([128, 2, NUM_WIN * BC], F16, tag="h")
                nc.vector.tensor_mul(hn, sifo[:, 4:6, :], tcn)  # o*tanh(c)
                nc.vector.tensor_add(pooled, pooled, hn)
                h_prev, c_prev = hn, cn

            # ---- FC ------------------------------------------------------
            fps = fc_ps.tile([CLS, BC], F32, tag="fc")
            for idx, (cw, k) in enumerate([(0, 0), (0, 1), (1, 0), (1, 1)]):
                nc.tensor.matmul(
                    out=fps,
                    lhsT=wfcT[:, idx * CLS:(idx + 1) * CLS],
                    rhs=pooled[:, k, cw * BC:(cw + 1) * BC],
                    start=(idx == 0),
                    stop=(idx == 3),
                )
            out_sb = persist.tile([CLS, BC], F32)
            nc.scalar.copy(out=out_sb, in_=fps)
            nc.sync.dma_start(out=out_d[:], in_=out_sb)

    nc.finalize()
    return nc


_CACHE = {}


def _get_nc(bias_zero: bool):
    if bias_zero not in _CACHE:
        _CACHE[bias_zero] = build(bias_zero)
    return _CACHE[bias_zero]


def kernel(x, W_ih, W_hh, b_ih, b_hh, W_fc, b_fc):
    from concourse.bass_utils import run_bass_kernel_spmd

    x = np.asarray(x, dtype=np.float32)
    W_ih = np.asarray(W_ih, dtype=np.float32)
    W_hh = np.asarray(W_hh, dtype=np.float32)
    b_ih = np.asarray(b_ih, dtype=np.float32)
    b_hh = np.asarray(b_hh, dtype=np.float32)
    W_fc = np.asarray(W_fc, dtype=np.float32)
    b_fc = np.asarray(b_fc, dtype=np.float32)

    bias = b_ih + b_hh
    bias_zero = bool(np.all(bias == 0.0))
    nc = _get_nc(bias_zero)

    in_maps = []
    for c in range(NCORES):
        xc = np.ascontiguousarray(
            x[c * BC:(c + 1) * BC].reshape(BC * T, C)
        )
        in_maps.append(
            {"x": xc, "w_ih": W_ih, "w_hh": W_hh, "w_fc": W_fc, "bias": bias}
        )

    res = run_bass_kernel_spmd(nc, in_maps, list(range(NCORES)))
    out = np.concatenate([r["out"].T for r in res.results], axis=0)
    return (out + b_fc[None, :]).astype(np.float32)



# revision 3
# speedup vs baseline: 1.0271x; 1.0271x over previous
"""Trainium2 Bass kernel for nn_Long_LSTM_Top (2-window masked LSTM + sum-pool + FC).

Strategy (hardcoded for B=256, T=300, C=128, H=256, CLS=60, windows at p=0 and
p=145, each 154 long, over the lag-1 difference sequence d[p] = x[p+1]-x[p]):

- Data-parallel over batch across 8 cores (32 batch rows/core).
- Per core, both windows' LSTM chains run fused: every tensor keeps the
  feature dim on partitions and (window, row) = 64 columns in the free dim,
  so the recurrence needs no transposes and each weight tile is loaded once
  per step for both windows.
- Per step the 8 gate chunks are split across TWO psum banks (one per
  H-half) so the activation tail of half x can start as soon as that bank's
  12 matmuls finish, instead of waiting for all 24 (psum bank-collision
  rule forces bank-granular read/write exclusion).
- Bank block order [g, f, i, o]: tanh(g) is block 0, sigmoid(f,i,o) is one
  contiguous 3-block activation. The c update uses a paired multiply:
  prod = [sf, si] * [c_prev, tanh_g] in one 128-col DVE op, then
  c_new = prod[0] + prod[1]. h = so*tanh(c) and the pooled accumulation
  run on the otherwise-idle gpsimd engine.
- Matmul operands fp16 (1 cycle/row on PE; fp32 would be 4), all elementwise
  state math fp32. Final FC in fp32.
"""

import numpy as np

import concourse.bass as bass
import concourse.mybir as mybir
from concourse import bacc
from concourse.tile import TileContext
from concourse.masks import make_identity

F32 = mybir.dt.float32
F16 = mybir.dt.float16

B, T, C, H, CLS = 256, 300, 128, 256, 60
START, STRIDE, WIN = 1, 145, 154
NUM_WIN = 2
L = T - START  # 299
NCORES = 8
BC = B // NCORES  # 32 rows per core
NSTEP = L  # 299 wall steps

# Bank block order [g, f, i, o]; PyTorch gate chunk index for (gate, half x):
# i -> 0+x, f -> 2+x, g -> 4+x, o -> 6+x.
BLK_GATE = [4, 2, 0, 6]  # chunk base per block position


def build(bias_zero: bool = True, nstep: int = NSTEP):
    """Build the per-core Bass module. Returns nc."""
    nc = bacc.Bacc("TRN2", target_bir_lowering=False, debug=False)

    x_d = nc.declare_dram_parameter("x", [BC * T, C], F32, isOutput=False)
    wih_d = nc.declare_dram_parameter("w_ih", [4 * H, C], F32, isOutput=False)
    whh_d = nc.declare_dram_parameter("w_hh", [4 * H, H], F32, isOutput=False)
    wfc_d = nc.declare_dram_parameter("w_fc", [CLS, NUM_WIN * H], F32, isOutput=False)
    bias_d = nc.declare_dram_parameter("bias", [4 * H], F32, isOutput=False)
    out_d = nc.declare_dram_parameter("out", [CLS, BC], F32, isOutput=True)

    with TileContext(nc) as tc:
        with (
            tc.tile_pool(name="persist", bufs=1) as persist,
            tc.tile_pool(name="prep", bufs=3) as prep,
            tc.tile_pool(name="prep_ps", bufs=2, space="PSUM") as prep_ps,
            tc.tile_pool(name="ps0", bufs=2, space="PSUM") as ps0_pool,
            tc.tile_pool(name="ps1", bufs=2, space="PSUM") as ps1_pool,
            tc.tile_pool(name="fc_ps", bufs=1, space="PSUM") as fc_ps,
            tc.tile_pool(name="sigp", bufs=3) as sigp,
            tc.tile_pool(name="ctgp", bufs=3) as ctgp,
            tc.tile_pool(name="prodp", bufs=3) as prodp,
            tc.tile_pool(name="tcp", bufs=3) as tcp,
            tc.tile_pool(name="hp", bufs=3) as hp,
        ):
            ident = persist.tile([128, 128], F32)
            make_identity(nc, ident)

            # ---- load x and transpose to xT[c, (r t)] --------------------
            xT = persist.tile([128, BC * T], F32)  # col = r*300 + t
            for j in range(75):
                xn = prep.tile([128, 128], F32, tag="xn")
                nc.sync.dma_start(out=xn, in_=x_d[j * 128:(j + 1) * 128, :])
                pt = prep_ps.tile([128, 128], F32)
                nc.tensor.transpose(pt, xn, ident)
                nc.scalar.copy(out=xT[:, j * 128:(j + 1) * 128], in_=pt)

            # ---- masked lag-difference, fp16, layout [c, (w win r)] ------
            dm = persist.tile([128, NSTEP, NUM_WIN, BC], F16)
            nc.vector.memset(dm, 0.0)
            xT3 = xT[:].rearrange("p (r t) -> p r t", r=BC)
            for r in range(BC):
                # window 0 active at p in [0, 154)
                nc.vector.tensor_sub(
                    dm[:, 0:WIN, 0, r],
                    xT3[:, r, 1:WIN + 1],
                    xT3[:, r, 0:WIN],
                )
                # window 1 active at p in [145, 299)
                nc.vector.tensor_sub(
                    dm[:, STRIDE:L, 1, r],
                    xT3[:, r, STRIDE + 1:L + 1],
                    xT3[:, r, STRIDE:L],
                )

            # ---- weights: transpose to [in_dim, gate] fp16 ---------------
            wihT = persist.tile([128, 8 * 128], F16)  # col block = gate chunk
            for g in range(8):
                wn = prep.tile([128, C], F32, tag="wn")
                nc.sync.dma_start(
                    out=wn, in_=wih_d[g * 128:(g + 1) * 128, :]
                )
                pt = prep_ps.tile([128, 128], F32)
                nc.tensor.transpose(pt, wn, ident)
                nc.scalar.copy(out=wihT[:, g * 128:(g + 1) * 128], in_=pt)

            whhT = persist.tile([128, 16 * 128], F16)  # col block = g*2+k
            for g in range(8):
                wn = prep.tile([128, H], F32, tag="wn2")
                nc.sync.dma_start(
                    out=wn, in_=whh_d[g * 128:(g + 1) * 128, :]
                )
                for k in range(2):
                    pt = prep_ps.tile([128, 128], F32)
                    nc.tensor.transpose(pt, wn[:, k * 128:(k + 1) * 128], ident)
                    nc.scalar.copy(
                        out=whhT[:, (g * 2 + k) * 128:(g * 2 + k + 1) * 128], in_=pt
                    )

            wfcT = persist.tile([128, 4 * CLS], F32)  # col block = feat chunk
            wfcn = persist.tile([CLS, NUM_WIN * H], F32)
            nc.sync.dma_start(out=wfcn, in_=wfc_d[:])
            for k in range(4):
                pt = prep_ps.tile([128, 128], F32)
                nc.tensor.transpose(
                    pt[:, :CLS], wfcn[:, k * 128:(k + 1) * 128], ident[:CLS, :CLS]
                )
                nc.scalar.copy(out=wfcT[:, k * CLS:(k + 1) * CLS], in_=pt[:, :CLS])

            bias_sb = None
            if not bias_zero:
                bias_sb = persist.tile([128, 8], F32)
                nc.sync.dma_start(
                    out=bias_sb, in_=bias_d[:].rearrange("(g p) -> p g", p=128)
                )

            # All prep (DMAs on many queues, transposes, masked-d subs) ends
            # here; without this barrier the first scan matmuls accumulate
            # more sync waits than the LDW ISA slot allows.
            tc.strict_bb_all_engine_barrier()

            # ---- scan ----------------------------------------------------
            pooled = persist.tile([128, 2, NUM_WIN * BC], F32)
            nc.vector.memset(pooled, 0.0)
            h_prev = hp.tile([128, 2, NUM_WIN * BC], F16, tag="h")
            nc.vector.memset(h_prev, 0.0)
            # ctg tile: per half x, [c_prev, tanh_g] pair blocks.
            ctg_cur = ctgp.tile([128, 2, 2, NUM_WIN * BC], F32, tag="ctg")
            nc.vector.memset(ctg_cur[:, :, 0, :], 0.0)

            sig = mybir.ActivationFunctionType.Sigmoid
            tnh = mybir.ActivationFunctionType.Tanh
            ps_pools = (ps0_pool, ps1_pool)

            for w in range(nstep):
                rhs_d = dm[:, w, :, :]
                ctg_next = ctgp.tile([128, 2, 2, NUM_WIN * BC], F32, tag="ctg")
                sg = sigp.tile([128, 2, 3, NUM_WIN * BC], F32, tag="sg")
                prod = prodp.tile([128, 2, 2, NUM_WIN * BC], F32, tag="pr")
                tcn = tcp.tile([128, 2, NUM_WIN * BC], F32, tag="tc")
                hn = hp.tile([128, 2, NUM_WIN * BC], F16, tag="h")
                banks = []
                for x in (0, 1):
                    ps = ps_pools[x].tile([128, 8, NUM_WIN * BC], F32, tag="ps")
                    banks.append(ps)
                    # Accumulation groups within one bank must be sequential
                    # (start=True zeroes the whole bank), so emit per block:
                    # [xproj, k_other, k_own].
                    korder = (1, 0) if x == 0 else (0, 1)
                    for b in range(4):
                        gc = BLK_GATE[b] + x
                        nc.tensor.matmul(
                            out=ps[:, b, :],
                            lhsT=wihT[:, gc * 128:(gc + 1) * 128],
                            rhs=rhs_d,
                            start=True,
                            stop=False,
                        )
                        for k in korder:
                            nc.tensor.matmul(
                                out=ps[:, b, :],
                                lhsT=whhT[:, (gc * 2 + k) * 128:(gc * 2 + k + 1) * 128],
                                rhs=h_prev[:, k, :],
                                start=False,
                                stop=(k == korder[-1]),
                            )

                for x in (0, 1):
                    ps = banks[x]
                    if bias_zero:
                        nc.scalar.activation(ctg_cur[:, x, 1, :], ps[:, 0, :], tnh)
                        nc.scalar.activation(sg[:, x, :, :], ps[:, 1:4, :], sig)
                    else:
                        nc.scalar.activation(
                            ctg_cur[:, x, 1, :], ps[:, 0, :], tnh,
                            bias=bias_sb[:, BLK_GATE[0] + x:BLK_GATE[0] + x + 1],
                        )
                        for b in range(1, 4):
                            nc.scalar.activation(
                                sg[:, x, b - 1, :], ps[:, b, :], sig,
                                bias=bias_sb[:, BLK_GATE[b] + x:BLK_GATE[b] + x + 1],
                            )
                    # prod = [sf*c_prev, si*tanh_g]
                    nc.vector.tensor_mul(
                        prod[:, x, :, :], sg[:, x, 0:2, :], ctg_cur[:, x, :, :]
                    )
                    nc.vector.tensor_add(
                        ctg_next[:, x, 0, :], prod[:, x, 0, :], prod[:, x, 1, :]
                    )
                    nc.scalar.activation(tcn[:, x, :], ctg_next[:, x, 0, :], tnh)
                    nc.gpsimd.tensor_mul(hn[:, x, :], sg[:, x, 2, :], tcn[:, x, :])

                nc.gpsimd.tensor_add(pooled, pooled, hn)
                h_prev = hn
                ctg_cur = ctg_next

            # ---- FC ------------------------------------------------------
            fps = fc_ps.tile([CLS, BC], F32, tag="fc")
            for idx, (cw, k) in enumerate([(0, 0), (0, 1), (1, 0), (1, 1)]):
                nc.tensor.matmul(
                    out=fps,
                    lhsT=wfcT[:, idx * CLS:(idx + 1) * CLS],
                    rhs=pooled[:, k, cw * BC:(cw + 1) * BC],
                    start=(idx == 0),
                    stop=(idx == 3),
                )
            out_sb = persist.tile([CLS, BC], F32)
            nc.scalar.copy(out=out_sb, in_=fps)
            nc.sync.dma_start(out=out_d[:], in_=out_sb)

    nc.finalize()
    return nc


_CACHE = {}


def _get_nc(bias_zero: bool):
    if bias_zero not in _CACHE:
        _CACHE[bias_zero] = build(bias_zero)
    return _CACHE[bias_zero]


def kernel(x, W_ih, W_hh, b_ih, b_hh, W_fc, b_fc):
    from concourse.bass_utils import run_bass_kernel_spmd

    x = np.asarray(x, dtype=np.float32)
    W_ih = np.asarray(W_ih, dtype=np.float32)
    W_hh = np.asarray(W_hh, dtype=np.float32)
    b_ih = np.asarray(b_ih, dtype=np.float32)
    b_hh = np.asarray(b_hh, dtype=np.float32)
    W_fc = np.asarray(W_fc, dtype=np.float32)
    b_fc = np.asarray(b_fc, dtype=np.float32)

    bias = b_ih + b_hh
    bias_zero = bool(np.all(bias == 0.0))
    nc = _get_nc(bias_zero)

    in_maps = []
    for c in range(NCORES):
        xc = np.ascontiguousarray(
            x[c * BC:(c + 1) * BC].reshape(BC * T, C)
        )
        in_maps.append(
            {"x": xc, "w_ih": W_ih, "w_hh": W_hh, "w_fc": W_fc, "bias": bias}
        )

    res = run_bass_kernel_spmd(nc, in_maps, list(range(NCORES)))
    out = np.concatenate([r["out"].T for r in res.results], axis=0)
    return (out + b_fc[None, :]).astype(np.float32)


# revision 6
# speedup vs baseline: 1.1356x; 1.1056x over previous
"""Trainium2 Bass kernel for nn_Long_LSTM_Top (2-window masked LSTM + sum-pool + FC).

Strategy (hardcoded for B=256, T=300, C=128, H=256, CLS=60, windows at p=0 and
p=145, each 154 long, over the lag-1 difference sequence d[p] = x[p+1]-x[p]):

- Data-parallel over batch across 8 cores (32 batch rows/core).
- Per core, both windows' LSTM chains run fused: every tensor keeps the
  feature dim on partitions and (window, row) = 64 columns in the free dim,
  so the recurrence needs no transposes and each weight tile is loaded once
  per step for both windows.
- Per step the 8 gate chunks are split across TWO psum banks (one per
  H-half) so the activation tail of half x can start as soon as that bank's
  12 matmuls finish, instead of waiting for all 24 (psum bank-collision
  rule forces bank-granular read/write exclusion).
- Bank block order [g, f, i, o]: tanh(g) is block 0, sigmoid(f,i,o) is one
  contiguous 3-block activation. The c update uses a paired multiply:
  prod = [sf, si] * [c_prev, tanh_g] in one 128-col DVE op, then
  c_new = prod[0] + prod[1]. h = so*tanh(c) and the pooled accumulation
  run on the otherwise-idle gpsimd engine.
- Matmul operands fp16 (1 cycle/row on PE; fp32 would be 4), all elementwise
  state math fp32. Final FC in fp32.
"""

import numpy as np

import concourse.bass as bass
import concourse.mybir as mybir
from concourse import bacc
from concourse.tile import TileContext
from concourse.masks import make_identity

F32 = mybir.dt.float32
F16 = mybir.dt.float16

B, T, C, H, CLS = 256, 300, 128, 256, 60
START, STRIDE, WIN = 1, 145, 154
NUM_WIN = 2
L = T - START  # 299
NCORES = 8
BC = B // NCORES  # 32 rows per core
NSTEP = L  # 299 wall steps

# Bank block order [g, f, i, o]; PyTorch gate chunk index for (gate, half x):
# i -> 0+x, f -> 2+x, g -> 4+x, o -> 6+x.
BLK_GATE = [4, 2, 0, 6]  # chunk base per block position


def build(bias_zero: bool = True, nstep: int = NSTEP):
    """Build the per-core Bass module. Returns nc."""
    nc = bacc.Bacc("TRN2", target_bir_lowering=False, debug=False)

    x_d = nc.declare_dram_parameter("x", [BC * T, C], F32, isOutput=False)
    wih_d = nc.declare_dram_parameter("w_ih", [4 * H, C], F32, isOutput=False)
    whh_d = nc.declare_dram_parameter("w_hh", [4 * H, H], F32, isOutput=False)
    wfc_d = nc.declare_dram_parameter("w_fc", [CLS, NUM_WIN * H], F32, isOutput=False)
    bias_d = nc.declare_dram_parameter("bias", [4 * H], F32, isOutput=False)
    out_d = nc.declare_dram_parameter("out", [CLS, BC], F32, isOutput=True)

    with TileContext(nc) as tc:
        with (
            tc.tile_pool(name="persist", bufs=1) as persist,
            tc.tile_pool(name="prep", bufs=3) as prep,
            tc.tile_pool(name="prep_ps", bufs=2, space="PSUM") as prep_ps,
            tc.tile_pool(name="ps0", bufs=2, space="PSUM") as ps0_pool,
            tc.tile_pool(name="ps1", bufs=2, space="PSUM") as ps1_pool,
            tc.tile_pool(name="fc_ps", bufs=1, space="PSUM") as fc_ps,
            tc.tile_pool(name="sigp", bufs=3) as sigp,
            tc.tile_pool(name="ctgp", bufs=3) as ctgp,
            tc.tile_pool(name="prodp", bufs=3) as prodp,
            tc.tile_pool(name="tcp", bufs=3) as tcp,
            tc.tile_pool(name="hp", bufs=3) as hp,
        ):
            ident = persist.tile([128, 128], F32)
            make_identity(nc, ident)

            # ---- load x and transpose to xT[c, (r t)] --------------------
            xT = persist.tile([128, BC * T], F32)  # col = r*300 + t
            for j in range(75):
                xn = prep.tile([128, 128], F32, tag="xn")
                nc.sync.dma_start(out=xn, in_=x_d[j * 128:(j + 1) * 128, :])
                pt = prep_ps.tile([128, 128], F32)
                nc.tensor.transpose(pt, xn, ident)
                nc.scalar.copy(out=xT[:, j * 128:(j + 1) * 128], in_=pt)

            # ---- masked lag-difference, fp16, layout [c, (w win r)] ------
            dm = persist.tile([128, NSTEP, NUM_WIN, BC], F16)
            nc.vector.memset(dm, 0.0)
            xT3 = xT[:].rearrange("p (r t) -> p r t", r=BC)
            for r in range(BC):
                # window 0 active at p in [0, 154)
                nc.vector.tensor_sub(
                    dm[:, 0:WIN, 0, r],
                    xT3[:, r, 1:WIN + 1],
                    xT3[:, r, 0:WIN],
                )
                # window 1 active at p in [145, 299)
                nc.vector.tensor_sub(
                    dm[:, STRIDE:L, 1, r],
                    xT3[:, r, STRIDE + 1:L + 1],
                    xT3[:, r, STRIDE:L],
                )

            # ---- weights: transpose to [in_dim, gate] fp16 ---------------
            wihT = persist.tile([128, 8 * 128], F16)  # col block = gate chunk
            for g in range(8):
                wn = prep.tile([128, C], F32, tag="wn")
                nc.sync.dma_start(
                    out=wn, in_=wih_d[g * 128:(g + 1) * 128, :]
                )
                pt = prep_ps.tile([128, 128], F32)
                nc.tensor.transpose(pt, wn, ident)
                nc.scalar.copy(out=wihT[:, g * 128:(g + 1) * 128], in_=pt)

            whhT = persist.tile([128, 16 * 128], F16)  # col block = g*2+k
            for g in range(8):
                wn = prep.tile([128, H], F32, tag="wn2")
                nc.sync.dma_start(
                    out=wn, in_=whh_d[g * 128:(g + 1) * 128, :]
                )
                for k in range(2):
                    pt = prep_ps.tile([128, 128], F32)
                    nc.tensor.transpose(pt, wn[:, k * 128:(k + 1) * 128], ident)
                    nc.scalar.copy(
                        out=whhT[:, (g * 2 + k) * 128:(g * 2 + k + 1) * 128], in_=pt
                    )

            wfcT = persist.tile([128, 4 * CLS], F32)  # col block = feat chunk
            wfcn = persist.tile([CLS, NUM_WIN * H], F32)
            nc.sync.dma_start(out=wfcn, in_=wfc_d[:])
            for k in range(4):
                pt = prep_ps.tile([128, 128], F32)
                nc.tensor.transpose(
                    pt[:, :CLS], wfcn[:, k * 128:(k + 1) * 128], ident[:CLS, :CLS]
                )
                nc.scalar.copy(out=wfcT[:, k * CLS:(k + 1) * CLS], in_=pt[:, :CLS])

            bias_sb = None
            if not bias_zero:
                bias_sb = persist.tile([128, 8], F32)
                nc.sync.dma_start(
                    out=bias_sb, in_=bias_d[:].rearrange("(g p) -> p g", p=128)
                )

            # All prep (DMAs on many queues, transposes, masked-d subs) ends
            # here; without this barrier the first scan matmuls accumulate
            # more sync waits than the LDW ISA slot allows.
            tc.strict_bb_all_engine_barrier()

            # ---- scan ----------------------------------------------------
            pooled = persist.tile([128, 2, NUM_WIN * BC], F32)
            nc.vector.memset(pooled, 0.0)
            h_prev = hp.tile([128, 2, NUM_WIN * BC], F16, tag="h")
            nc.vector.memset(h_prev, 0.0)
            # ctg tile: per half x, [c_prev, tanh_g] pair blocks.
            ctg_cur = ctgp.tile([128, 2, 2, NUM_WIN * BC], F32, tag="ctg")
            nc.vector.memset(ctg_cur[:, :, 0, :], 0.0)

            sig = mybir.ActivationFunctionType.Sigmoid
            tnh = mybir.ActivationFunctionType.Tanh
            ps_pools = (ps0_pool, ps1_pool)

            for w in range(nstep):
                rhs_d = dm[:, w, :, :]
                ctg_next = ctgp.tile([128, 2, 2, NUM_WIN * BC], F32, tag="ctg")
                sg = sigp.tile([128, 2, 4, NUM_WIN * BC], F32, tag="sg")
                prod = prodp.tile([128, 2, 2, NUM_WIN * BC], F32, tag="pr")
                tcn = tcp.tile([128, 2, NUM_WIN * BC], F32, tag="tc")
                hn = hp.tile([128, 2, NUM_WIN * BC], F16, tag="h")
                banks = []
                for x in (0, 1):
                    ps = ps_pools[x].tile([128, 8, NUM_WIN * BC], F32, tag="ps")
                    banks.append(ps)
                    # Accumulation groups within one bank must be sequential
                    # (start=True zeroes the whole bank), so emit per block:
                    # [xproj, k_other, k_own].
                    korder = (1, 0) if x == 0 else (0, 1)
                    for b in range(4):
                        gc = BLK_GATE[b] + x
                        nc.tensor.matmul(
                            out=ps[:, b, :],
                            lhsT=wihT[:, gc * 128:(gc + 1) * 128],
                            rhs=rhs_d,
                            start=True,
                            stop=False,
                        )
                        for k in korder:
                            nc.tensor.matmul(
                                out=ps[:, b, :],
                                lhsT=whhT[:, (gc * 2 + k) * 128:(gc * 2 + k + 1) * 128],
                                rhs=h_prev[:, k, :],
                                start=False,
                                stop=(k == korder[-1]),
                            )

                # One sigmoid per bank covers all 4 gates: g rows were doubled
                # host-side, so block 0 holds sigma(2g) and tanh(g) = 2*sg-1.
                for x in (0, 1):
                    ps = banks[x]
                    if bias_zero:
                        nc.scalar.activation(sg[:, x, :, :, ], ps[:, 0:4, :], sig)
                    else:
                        for b in range(4):
                            nc.scalar.activation(
                                sg[:, x, b, :], ps[:, b, :], sig,
                                bias=bias_sb[:, BLK_GATE[b] + x:BLK_GATE[b] + x + 1],
                            )
                for x in (0, 1):
                    nc.vector.tensor_scalar(
                        ctg_cur[:, x, 1, :], sg[:, x, 0, :], 2.0, -1.0,
                        mybir.AluOpType.mult, mybir.AluOpType.add,
                    )
                    # prod = [sf*c_prev, si*tanh_g]
                    nc.vector.tensor_mul(
                        prod[:, x, :, :], sg[:, x, 1:3, :], ctg_cur[:, x, :, :]
                    )
                    nc.vector.tensor_add(
                        ctg_next[:, x, 0, :], prod[:, x, 0, :], prod[:, x, 1, :]
                    )
                for x in (0, 1):
                    nc.scalar.activation(tcn[:, x, :], ctg_next[:, x, 0, :], tnh)
                    nc.gpsimd.tensor_mul(hn[:, x, :], sg[:, x, 3, :], tcn[:, x, :])

                nc.gpsimd.tensor_add(pooled, pooled, hn)
                h_prev = hn
                ctg_cur = ctg_next

            # ---- FC ------------------------------------------------------
            fps = fc_ps.tile([CLS, BC], F32, tag="fc")
            for idx, (cw, k) in enumerate([(0, 0), (0, 1), (1, 0), (1, 1)]):
                nc.tensor.matmul(
                    out=fps,
                    lhsT=wfcT[:, idx * CLS:(idx + 1) * CLS],
                    rhs=pooled[:, k, cw * BC:(cw + 1) * BC],
                    start=(idx == 0),
                    stop=(idx == 3),
                )
            out_sb = persist.tile([CLS, BC], F32)
            nc.scalar.copy(out=out_sb, in_=fps)
            nc.sync.dma_start(out=out_d[:], in_=out_sb)

    nc.finalize()
    return nc


_CACHE = {}


def _get_nc(bias_zero: bool):
    if bias_zero not in _CACHE:
        _CACHE[bias_zero] = build(bias_zero)
    return _CACHE[bias_zero]


def kernel(x, W_ih, W_hh, b_ih, b_hh, W_fc, b_fc):
    from concourse.bass_utils import run_bass_kernel_spmd

    x = np.asarray(x, dtype=np.float32)
    W_ih = np.asarray(W_ih, dtype=np.float32)
    W_hh = np.asarray(W_hh, dtype=np.float32)
    b_ih = np.asarray(b_ih, dtype=np.float32)
    b_hh = np.asarray(b_hh, dtype=np.float32)
    W_fc = np.asarray(W_fc, dtype=np.float32)
    b_fc = np.asarray(b_fc, dtype=np.float32)

    bias = b_ih + b_hh
    bias_zero = bool(np.all(bias == 0.0))
    nc = _get_nc(bias_zero)

    # Fold tanh(g) = 2*sigmoid(2g) - 1 into the matmul: double the g-gate
    # rows (PyTorch chunk 2 of 4) of both weight matrices and the bias.
    W_ih = W_ih.copy()
    W_hh = W_hh.copy()
    bias = bias.copy()
    W_ih[2 * H:3 * H] *= 2.0
    W_hh[2 * H:3 * H] *= 2.0
    bias[2 * H:3 * H] *= 2.0

    in_maps = []
    for c in range(NCORES):
        xc = np.ascontiguousarray(
            x[c * BC:(c + 1) * BC].reshape(BC * T, C)
        )
        in_maps.append(
            {"x": xc, "w_ih": W_ih, "w_hh": W_hh, "w_fc": W_fc, "bias": bias}
        )

    res = run_bass_kernel_spmd(nc, in_maps, list(range(NCORES)))
    out = np.concatenate([r["out"].T for r in res.results], axis=0)
    return (out + b_fc[None, :]).astype(np.float32)


# revision 12
# speedup vs baseline: 1.3228x; 1.1648x over previous
"""Trainium2 Bass kernel for nn_Long_LSTM_Top (2-window masked LSTM + sum-pool + FC).

Strategy (hardcoded for B=256, T=300, C=128, H=256, CLS=60, windows at p=0 and
p=145, each 154 long, over the lag-1 difference sequence d[p] = x[p+1]-x[p]):

- Data-parallel over batch across 8 cores (32 batch rows/core).
- Per core, both windows' LSTM chains run fused: every tensor keeps the
  feature dim on partitions and (row, window) = 64 columns in the free dim,
  so the recurrence needs no transposes and each weight tile is loaded once
  per step for both windows.
- Per step the 8 gate chunks are split across TWO psum banks (one per
  H-half) so the activation tail of half x can start as soon as that bank's
  matmuls finish (psum bank-collision rule forces bank-granular exclusion).
- The 8 input-projection matmuls lead their banks (first with start=True to
  clear the bank, the rest start=False, overwriting where has_written is
  clear) so they hoist out of the h-dependent critical burst; only the 16
  W_hh matmuls sit between h(t-1) and the sigmoids.
- tanh(g) is folded into the matmul: g-gate weight rows are doubled on the
  host, one sigmoid per bank covers all 4 gates, tanh(g) = 2*sigma(2g)-1 is
  a vector op. The c update uses a paired multiply: [sf, si] * [c, tanh_g]
  in one DVE op + one pair-add. h = so*tanh(c) runs on gpsimd (half 0) and
  DVE (half 1); pooled accumulates on gpsimd.
- Matmul operands fp16, all elementwise state math fp32. Final FC in fp32.
"""

import numpy as np

import concourse.bass as bass
import concourse.mybir as mybir
from concourse import bacc
from concourse.tile import TileContext
from concourse.masks import make_identity

F32 = mybir.dt.float32
F16 = mybir.dt.float16

B, T, C, H, CLS = 256, 300, 128, 256, 60
START, STRIDE, WIN = 1, 145, 154
NUM_WIN = 2
L = T - START  # 299
NCORES = 8
BC = B // NCORES  # 32 rows per core
NSTEP = L  # 299 wall steps
NCOL = NUM_WIN * BC  # 64 scan columns, (row, window) order

# Bank block order [g, f, i, o]; PyTorch gate chunk index for (gate, half x):
# i -> 0+x, f -> 2+x, g -> 4+x, o -> 6+x.
BLK_GATE = [4, 2, 0, 6]


def build(bias_zero: bool = True, nstep: int = NSTEP):
    """Build the per-core Bass module. Returns nc."""
    nc = bacc.Bacc("TRN2", target_bir_lowering=False, debug=False)

    x_d = nc.declare_dram_parameter("x", [BC * T, C], F32, isOutput=False)
    wih_d = nc.declare_dram_parameter("w_ih", [4 * H, C], F32, isOutput=False)
    whh_d = nc.declare_dram_parameter("w_hh", [4 * H, H], F32, isOutput=False)
    wfc_d = nc.declare_dram_parameter("w_fc", [CLS, NUM_WIN * H], F32, isOutput=False)
    bias_d = nc.declare_dram_parameter("bias", [4 * H], F32, isOutput=False)
    out_d = nc.declare_dram_parameter("out", [CLS, BC], F32, isOutput=True)

    with TileContext(nc) as tc:
        with (
            tc.tile_pool(name="persist", bufs=1) as persist,
            tc.tile_pool(name="prep", bufs=4) as prep,
            tc.tile_pool(name="prep_ps", bufs=2, space="PSUM") as prep_ps,
            tc.tile_pool(name="ps0", bufs=2, space="PSUM") as ps0_pool,
            tc.tile_pool(name="ps1", bufs=2, space="PSUM") as ps1_pool,
            tc.tile_pool(name="fc_ps", bufs=1, space="PSUM") as fc_ps,
            tc.tile_pool(name="sigp", bufs=3) as sigp,
            tc.tile_pool(name="ctgp", bufs=3) as ctgp,
            tc.tile_pool(name="prodp", bufs=3) as prodp,
            tc.tile_pool(name="tcp", bufs=3) as tcp,
            tc.tile_pool(name="hp", bufs=3) as hp,
        ):
            ident = persist.tile([128, 128], F32)
            make_identity(nc, ident)

            # ---- weights first (the scan needs them before anything) -----
            # One big strided DMA per tensor: row g*128+p -> partition p,
            # column block g. Small DMAs serialize ~1us each on the sync
            # queue, so batch aggressively.
            whraw = persist.tile([128, 8 * H], F32)
            nc.sync.dma_start(
                out=whraw[:].rearrange("p (g h) -> p g h", g=8),
                in_=whh_d[:].rearrange("(g p) h -> p g h", p=128),
            )
            whhT = persist.tile([128, 16 * 128], F16)  # col block = g*2+k
            for batch in range(4):  # 4 transposes per psum bank
                pt = prep_ps.tile([128, 512], F32, tag="wps")
                for j in range(4):
                    gk = batch * 4 + j  # g*2+k
                    g, k = gk // 2, gk % 2
                    nc.tensor.transpose(
                        pt[:, j * 128:(j + 1) * 128],
                        whraw[:, (g * 2 + k) * 128:(g * 2 + k + 1) * 128], ident,
                    )
                if batch % 2 == 0:
                    nc.scalar.copy(out=whhT[:, batch * 512:(batch + 1) * 512], in_=pt)
                else:
                    nc.vector.tensor_copy(whhT[:, batch * 512:(batch + 1) * 512], pt)

            wiraw = persist.tile([128, 8 * C], F32)
            nc.sync.dma_start(
                out=wiraw[:].rearrange("p (g c) -> p g c", g=8),
                in_=wih_d[:].rearrange("(g p) c -> p g c", p=128),
            )
            wihT = persist.tile([128, 8 * 128], F16)  # col block = gate chunk
            for batch in range(2):
                pt = prep_ps.tile([128, 512], F32, tag="wps")
                for j in range(4):
                    g = batch * 4 + j
                    nc.tensor.transpose(
                        pt[:, j * 128:(j + 1) * 128],
                        wiraw[:, g * 128:(g + 1) * 128], ident,
                    )
                if batch % 2 == 0:
                    nc.scalar.copy(out=wihT[:, batch * 512:(batch + 1) * 512], in_=pt)
                else:
                    nc.vector.tensor_copy(wihT[:, batch * 512:(batch + 1) * 512], pt)

            # ---- load x with 4 big DMAs, transpose to xT[c, (r t)] -------
            xraw = persist.tile([128, 75 * 128], F32)
            XCHUNK = [20, 20, 20, 15]  # 128-row blocks per DMA
            off = 0
            for nblk in XCHUNK:
                nc.sync.dma_start(
                    out=xraw[:, off * 128:(off + nblk) * 128].rearrange(
                        "p (f c) -> p f c", c=C
                    ),
                    in_=x_d[off * 128:(off + nblk) * 128, :].rearrange(
                        "(f p) c -> p f c", p=128
                    ),
                )
                off += nblk
            xT = persist.tile([128, BC * T], F32)  # col = r*300 + t
            for batch in range(19):
                nblk = 4 if batch < 18 else 3
                pt = prep_ps.tile([128, 512], F32, tag="wps")
                for j in range(nblk):
                    blk = batch * 4 + j
                    nc.tensor.transpose(
                        pt[:, j * 128:(j + 1) * 128],
                        xraw[:, blk * 128:(blk + 1) * 128], ident,
                    )
                if batch % 2 == 0:
                    nc.scalar.copy(
                        out=xT[:, batch * 512:batch * 512 + nblk * 128],
                        in_=pt[:, :nblk * 128],
                    )
                else:
                    nc.vector.tensor_copy(
                        xT[:, batch * 512:batch * 512 + nblk * 128],
                        pt[:, :nblk * 128],
                    )

            # ---- masked lag-difference, fp16, layout [c, r, win, t] ------
            dm = persist.tile([128, BC, NUM_WIN, NSTEP], F16)
            nc.gpsimd.memset(dm, 0.0)
            xT3 = xT[:].rearrange("p (r t) -> p r t", r=BC)
            # window 0 active at p in [0, 154); window 1 at p in [145, 299).
            # Two pieces per window so the scan can start after piece one.
            for (wwin, lo, hi) in [(0, 0, 80), (1, STRIDE, 224), (0, 80, WIN),
                                   (1, 224, L)]:
                nc.vector.tensor_sub(
                    dm[:, :, wwin, lo:hi],
                    xT3[:, :, lo + 1:hi + 1],
                    xT3[:, :, lo:hi],
                )

            wfcT = persist.tile([128, 4 * CLS], F32)  # col block = feat chunk
            wfcn = persist.tile([CLS, NUM_WIN * H], F32)
            nc.sync.dma_start(out=wfcn, in_=wfc_d[:])
            pt = prep_ps.tile([128, 512], F32, tag="wps")
            for k in range(4):
                nc.tensor.transpose(
                    pt[:, k * 128:k * 128 + CLS],
                    wfcn[:, k * 128:(k + 1) * 128], ident[:CLS, :CLS],
                )
            for k in range(4):
                nc.scalar.copy(
                    out=wfcT[:, k * CLS:(k + 1) * CLS],
                    in_=pt[:, k * 128:k * 128 + CLS],
                )

            bias_sb = None
            if not bias_zero:
                bias_sb = persist.tile([128, 8], F32)
                nc.sync.dma_start(
                    out=bias_sb, in_=bias_d[:].rearrange("(g p) -> p g", p=128)
                )

            # ---- scan ----------------------------------------------------
            pooled = persist.tile([128, 2, NCOL], F32)
            nc.gpsimd.memset(pooled, 0.0)
            h_prev = hp.tile([128, 2, NCOL], F16, tag="h")
            nc.gpsimd.memset(h_prev, 0.0)
            # ctg tile: per half x, [c_prev, tanh_g] pair blocks.
            ctg_cur = ctgp.tile([128, 2, 2, NCOL], F32, tag="ctg")
            nc.gpsimd.memset(ctg_cur[:, :, 0, :], 0.0)

            sig = mybir.ActivationFunctionType.Sigmoid
            tnh = mybir.ActivationFunctionType.Tanh
            ps_pools = (ps0_pool, ps1_pool)

            for w in range(nstep):
                rhs_d = dm[:, :, :, w]
                ctg_next = ctgp.tile([128, 2, 2, NCOL], F32, tag="ctg")
                sg = sigp.tile([128, 2, 4, NCOL], F32, tag="sg")
                prod = prodp.tile([128, 2, 2, NCOL], F32, tag="pr")
                tcn = tcp.tile([128, 2, NCOL], F32, tag="tc")
                hn = hp.tile([128, 2, NCOL], F16, tag="h")
                banks = []
                # Input-projection matmuls lead each bank: the first clears
                # the bank (start=True), the rest overwrite fresh regions
                # (start=False with has_written clear), so all 8 can run
                # before h(t-1) exists.
                for x in (0, 1):
                    ps = ps_pools[x].tile([128, 8, NCOL], F32, tag="ps")
                    banks.append(ps)
                    for b in range(4):
                        gc = BLK_GATE[b] + x
                        nc.tensor.matmul(
                            out=ps[:, b, :],
                            lhsT=wihT[:, gc * 128:(gc + 1) * 128],
                            rhs=rhs_d,
                            start=(b == 0),
                            stop=False,
                            skip_group_check=True,
                        )
                # W_hh partials: all k0 (need only h half 0, which lands one
                # pipeline stage before half 1) then all k1.
                for k in (0, 1):
                    for x in (0, 1):
                        ps = banks[x]
                        for b in range(4):
                            gc = BLK_GATE[b] + x
                            nc.tensor.matmul(
                                out=ps[:, b, :],
                                lhsT=whhT[:, (gc * 2 + k) * 128:(gc * 2 + k + 1) * 128],
                                rhs=h_prev[:, k, :],
                                start=False,
                                stop=(k == 1),
                                skip_group_check=True,
                            )
                # One sigmoid per bank covers all 4 gates: g rows were doubled
                # host-side, so block 0 holds sigma(2g) and tanh(g) = 2*sg-1.
                for x in (0, 1):
                    ps = banks[x]
                    if bias_zero:
                        nc.scalar.activation(sg[:, x, :, :], ps[:, 0:4, :], sig)
                    else:
                        for b in range(4):
                            nc.scalar.activation(
                                sg[:, x, b, :], ps[:, b, :], sig,
                                bias=bias_sb[:, BLK_GATE[b] + x:BLK_GATE[b] + x + 1],
                            )
                for x in (0, 1):
                    nc.vector.tensor_scalar(
                        ctg_cur[:, x, 1, :], sg[:, x, 0, :], 2.0, -1.0,
                        mybir.AluOpType.mult, mybir.AluOpType.add,
                    )
                    # prod = [sf*c_prev, si*tanh_g]
                    nc.vector.tensor_mul(
                        prod[:, x, :, :], sg[:, x, 1:3, :], ctg_cur[:, x, :, :]
                    )
                    nc.vector.tensor_add(
                        ctg_next[:, x, 0, :], prod[:, x, 0, :], prod[:, x, 1, :]
                    )
                nc.scalar.activation(tcn[:, 0, :], ctg_next[:, 0, 0, :], tnh)
                nc.gpsimd.tensor_mul(hn[:, 0, :], sg[:, 0, 3, :], tcn[:, 0, :])
                nc.scalar.activation(tcn[:, 1, :], ctg_next[:, 1, 0, :], tnh)
                nc.vector.tensor_mul(hn[:, 1, :], sg[:, 1, 3, :], tcn[:, 1, :])

                nc.gpsimd.tensor_add(pooled, pooled, hn)
                h_prev = hn
                ctg_cur = ctg_next

            # ---- FC ------------------------------------------------------
            # pooled columns are (row, window)-ordered; FC consumes
            # window-major slices via a strided AP (one-shot, cost ok).
            pooled4 = pooled[:].rearrange("p k (r v) -> p k r v", v=NUM_WIN)
            fps = fc_ps.tile([CLS, BC], F32, tag="fc")
            for idx, (cw, k) in enumerate([(0, 0), (0, 1), (1, 0), (1, 1)]):
                nc.tensor.matmul(
                    out=fps,
                    lhsT=wfcT[:, idx * CLS:(idx + 1) * CLS],
                    rhs=pooled4[:, k, :, cw],
                    start=(idx == 0),
                    stop=(idx == 3),
                )
            out_sb = persist.tile([CLS, BC], F32)
            nc.scalar.copy(out=out_sb, in_=fps)
            nc.sync.dma_start(out=out_d[:], in_=out_sb)

    nc.finalize()
    return nc


_CACHE = {}


def _get_nc(bias_zero: bool):
    if bias_zero not in _CACHE:
        _CACHE[bias_zero] = build(bias_zero)
    return _CACHE[bias_zero]


def kernel(x, W_ih, W_hh, b_ih, b_hh, W_fc, b_fc):
    from concourse.bass_utils import run_bass_kernel_spmd

    x = np.asarray(x, dtype=np.float32)
    W_ih = np.asarray(W_ih, dtype=np.float32)
    W_hh = np.asarray(W_hh, dtype=np.float32)
    b_ih = np.asarray(b_ih, dtype=np.float32)
    b_hh = np.asarray(b_hh, dtype=np.float32)
    W_fc = np.asarray(W_fc, dtype=np.float32)
    b_fc = np.asarray(b_fc, dtype=np.float32)

    bias = b_ih + b_hh
    bias_zero = bool(np.all(bias == 0.0))
    nc = _get_nc(bias_zero)

    # Fold tanh(g) = 2*sigmoid(2g) - 1 into the matmul: double the g-gate
    # rows (PyTorch chunk 2 of 4) of both weight matrices and the bias.
    W_ih = W_ih.copy()
    W_hh = W_hh.copy()
    bias = bias.copy()
    W_ih[2 * H:3 * H] *= 2.0
    W_hh[2 * H:3 * H] *= 2.0
    bias[2 * H:3 * H] *= 2.0

    in_maps = []
    for c in range(NCORES):
        xc = np.ascontiguousarray(
            x[c * BC:(c + 1) * BC].reshape(BC * T, C)
        )
        in_maps.append(
            {"x": xc, "w_ih": W_ih, "w_hh": W_hh, "w_fc": W_fc, "bias": bias}
        )

    res = run_bass_kernel_spmd(nc, in_maps, list(range(NCORES)))
    out = np.concatenate([r["out"].T for r in res.results], axis=0)
    return (out + b_fc[None, :]).astype(np.float32)
